# revision 2
# baseline (speedup 1.0000x reference)
"""Block-sparse linear kernel for Trainium2 (8 NeuronCores) — quadrant design.

Computes: mask = mean|x| per 64x64 block > 0.798; out = (x*mask) @ weight
for x [4096,4096] f32, weight [4096,11008] f32 -> out [4096,11008] f32.

Strategy (v2 — PE 2x2 array tiling):
- Weight column-sharded across 8 cores (1376 cols each); x replicated.
- The PE array is addressed as four 64x64 quadrants via tile_position:
  row-half r = k-block parity (fixed by the w SBUF layout), col-half c =
  output chunk-group. N=1376 is split into 4 chunks of 344; quadrant
  (r, c) streams chunks {2c, 2c+1}. Each live (m,k) cell is ONE
  independent unit of work: 4 matmuls (2 per col-quadrant) sharing one
  [64,64] stationary x block. No m-block pairing, no union padding —
  every streamed column is live work, which is what beats the
  union-packed baseline (699 steps -> ~1003 cell-steps of half width).
- PSUM: per m-block, 8 accumulator half-banks (2 rows x 4 chunks) = 4
  banks; consecutive m-blocks flip-flop between bank sets {0..3} and
  {4..7}, so seat turnover never stalls the PE. Drains (ACT copy +
  DVE add per chunk; psum allows only one operand per instr) and 2
  output DMAs per m-block overlap the next block's accumulation.
- The E/O rows advance through the same m-order independently (kept
  within +-1 m by the bank flip-flop); per-m parity balance from the
  host-side assignment keeps that coupling cheap.
- bf16 inputs, fp32 PSUM accumulation. Host packs a compacted lhsT
  stream of live x blocks only (~16MB vs 25MB padded in v1).
"""

import numpy as np
import ml_dtypes

import concourse.bacc as bacc
import concourse.mybir as mybir
import concourse.tile as tile
from concourse.bass_utils import run_bass_kernel_spmd

M = 4096
K = 4096
N = 11008
B = 64
NB = M // B       # 64 blocks per dim
NCORES = 8
NSH = N // NCORES  # 1376 output cols per core
THRES = 0.798
CW = 344
CHUNKS = [(0, CW), (CW, CW), (2 * CW, CW), (3 * CW, CW)]
G = 32            # cells per x-stream DMA tile (per row-half)
BF16 = mybir.dt.bfloat16
F32 = mybir.dt.float32


def _block_mask(x):
    xb = np.abs(x.reshape(NB, B, NB, B))
    bm = xb.mean(axis=(1, 3), dtype=np.float64)
    return bm > THRES


def _parity_assign(mask):
    """Parity sig in {+-1} minimizing 8*|global E-O imbalance| (the wall
    time follows max(cellsE, cellsO)) + sum_m |ev_m - od_m| (keeps the two
    PE row-halves in lockstep per m)."""
    Mi = mask.astype(np.int32)
    rng = np.random.default_rng(1234)

    def cost(d):
        return np.abs(d).sum() + 8 * abs(int(d.sum()))

    best_sig, best_c = None, 1 << 30
    for _ in range(6):
        sig = np.array([1] * (NB // 2) + [-1] * (NB // 2))
        rng.shuffle(sig)
        improved = True
        while improved:
            improved = False
            d = Mi @ sig
            cur = cost(d)
            pos = np.where(sig == 1)[0]
            neg = np.where(sig == -1)[0]
            bestswap, bestdelta = None, 0
            for i in pos:
                for j in neg:
                    nd = d - 2 * Mi[:, i] + 2 * Mi[:, j]
                    delta = cost(nd) - cur
                    if delta < bestdelta:
                        bestdelta, bestswap = delta, (i, j)
            if bestswap is not None:
                i, j = bestswap
                sig[i], sig[j] = -1, 1
                improved = True
        c = cost(Mi @ sig)
        if c < best_c:
            best_c, best_sig = c, sig.copy()
    return (best_sig == -1).astype(np.int8)  # 1 = odd (array rows 64:128)


def _schedule(mask, par):
    """Per-m, per-parity k queues + w tile layout + m processing order."""
    evens = [b for b in range(NB) if par[b] == 0]
    odds = [b for b in range(NB) if par[b] == 1]
    assert len(evens) == len(odds) == NB // 2
    wloc = {}
    for t in range(NB // 2):
        wloc[evens[t]] = (t, 0)
        wloc[odds[t]] = (t, 1)
    morder = list(range(NB))
    qs = {}
    for m in range(NB):
        qs[(m, 0)] = [b for b in range(NB) if mask[m, b] and par[b] == 0]
        qs[(m, 1)] = [b for b in range(NB) if mask[m, b] and par[b] == 1]
    return qs, wloc, morder


def _emit_order(qs, morder):
    """Interleaved emission: alternate E/O cells, rows advance through the
    same m-order independently but stay within +-1 m (bank flip-flop).
    Returns: cells = list of (r, m, k, start, stop, slot) in program order,
    drains = {position_in_cells: [m, ...]} drains to emit after that cell,
    nslots = stream slots per row-half."""
    state = {r: {"mi": 0, "qi": 0} for r in (0, 1)}
    slots = {0: 0, 1: 0}
    done_m = {r: -1 for r in (0, 1)}  # last m-index fully emitted by row r
    cells = []
    drains = {}
    pend_drain = set()

    def row_can_emit(r):
        st = state[r]
        while st["mi"] < len(morder):
            m = morder[st["mi"]]
            q = qs[(m, r)]
            if st["qi"] < len(q):
                # bank constraint: row r may work on m-index i only if the
                # other row is at index >= i-1 (banks of i-2 must be free;
                # freed once both rows passed i-2 and drain emitted).
                other = 1 - r
                if st["mi"] - done_m[other] > 2:
                    return False
                return True
            done_m[r] = st["mi"]
            st["mi"] += 1
            st["qi"] = 0
            # m complete on this row; if complete on both -> drain
            if done_m[1 - r] >= st["mi"] - 1:
                pass
        return False

    def emit(r):
        st = state[r]
        m = morder[st["mi"]]
        q = qs[(m, r)]
        k = q[st["qi"]]
        start = st["qi"] == 0
        stop = st["qi"] == len(q) - 1
        cells.append((r, m, k, start, stop, slots[r]))
        slots[r] += 1
        st["qi"] += 1
        if stop:
            done_m[r] = st["mi"]
            st["mi"] += 1
            st["qi"] = 0
            # skip m's with an empty queue on this row
            while st["mi"] < len(morder) and not qs[(morder[st["mi"]], r)]:
                done_m[r] = st["mi"]
                st["mi"] += 1
        # emit drain for any m fully processed by both rows
        dm = min(done_m[0], done_m[1])
        for i in range(dm + 1):
            m2 = morder[i]
            if m2 not in pend_drain:
                pend_drain.add(m2)
                drains.setdefault(len(cells), []).append(m2)

    remaining = True
    turn = 0
    while remaining:
        e0 = row_can_emit(turn)
        e1 = row_can_emit(1 - turn)
        if e0:
            emit(turn)
        if e1:
            emit(1 - turn)
        if not e0 and not e1:
            if state[0]["mi"] >= len(morder) and state[1]["mi"] >= len(morder):
                remaining = False
            else:
                raise RuntimeError("emission deadlock")
        turn = 1 - turn
    # drain anything left
    tail = [m for m in morder if m not in pend_drain]
    if tail:
        drains.setdefault(len(cells), []).extend(tail)
    nslots = max(slots[0], slots[1])
    return cells, drains, nslots


def _pack_stream(x, qs, morder, nslots):
    """bf16 lhsT stream [128, S*64]: rows 0:64 = E cells (xT blocks in
    consumption order), rows 64:128 = O cells."""
    s_pad = ((nslots + G - 1) // G) * G
    xs = np.zeros((128, s_pad * 64), dtype=np.float32)
    for r in (0, 1):
        s = 0
        for m in morder:
            for k in qs[(m, r)]:
                blk = x[m * B:(m + 1) * B, k * B:(k + 1) * B]
                xs[64 * r:64 * r + 64, s * 64:(s + 1) * 64] = blk.T
                s += 1
    return xs.astype(ml_dtypes.bfloat16), s_pad


def _w_row_index(wloc):
    idx = np.empty(K, dtype=np.int64)
    for b, (t, r) in wloc.items():
        idx[128 * t + 64 * r: 128 * t + 64 * r + 64] = np.arange(b * B, (b + 1) * B)
    return idx


def _build(qs, wloc, morder, cells, drains, s_pad, reps=1):
    nc = bacc.Bacc()
    xs_d = nc.declare_dram_parameter("xs", [128, s_pad * 64], BF16, isOutput=False)
    w_d = nc.declare_dram_parameter("w", [K, NSH], BF16, isOutput=False)
    out_d = nc.declare_dram_parameter("out", [M, NSH], F32, isOutput=True)

    # w tile first-use order (preload in the order the schedule touches them)
    worder = []
    seen_t = set()
    for (r, m, k, st, sp, sl) in cells:
        t = wloc[k][0]
        if t not in seen_t:
            seen_t.add(t)
            worder.append(t)
    for t in range(NB // 2):
        if t not in seen_t:
            worder.append(t)

    mindex = {m: i for i, m in enumerate(morder)}

    with tile.TileContext(nc) as tc:
        with (
            tc.tile_pool(name="wp", bufs=1) as wp,
            tc.tile_pool(name="xp", bufs=6) as xp,
            tc.tile_pool(name="sp", bufs=6) as sp,
            tc.tile_pool(name="pp", bufs=1, space="PSUM") as pp,
        ):
            wts = {}
            for t in worder:
                wt = wp.tile([128, NSH], BF16, tag=f"w{t}", name=f"w{t}")
                nc.sync.dma_start(wt[:], w_d[128 * t:128 * (t + 1), :])
                wts[t] = wt
            banks = [
                pp.tile([128, 512], F32, tag=f"b{i}", bufs=1, name=f"bank{i}")
                for i in range(8)
            ]

            def bank_of(m, r, j):
                bs = 4 * (mindex[m] % 2)
                return banks[bs + 2 * r + (j & 1)]

            for _ in range(reps):
                xg_tiles = {}
                stages = {}

                def need_tile(slot):
                    g = slot // G
                    if g not in xg_tiles:
                        xg = xp.tile([128, G * 64], BF16, tag="xg", name="xg")
                        nc.sync.dma_start(
                            xg[:], xs_d[:, g * G * 64:(g + 1) * G * 64])
                        xg_tiles[g] = xg
                    return xg_tiles[g]

                def do_drain(m):
                    heE = len(qs[(m, 0)]) > 0
                    heO = len(qs[(m, 1)]) > 0
                    stage = sp.tile([128, NSH], F32, tag="stage", name="stage")
                    for j, (c0, cw) in enumerate(CHUNKS):
                        cc = j // 2
                        sl = slice(64 * cc, 64 * cc + 64)
                        if heE and heO:
                            nc.scalar.copy(
                                stage[sl, c0:c0 + cw], bank_of(m, 0, j)[sl, :cw])
                            nc.vector.tensor_tensor(
                                stage[sl, c0:c0 + cw],
                                bank_of(m, 1, j)[sl, :cw],
                                stage[sl, c0:c0 + cw],
                                mybir.AluOpType.add)
                        elif heE:
                            nc.vector.tensor_copy(
                                stage[sl, c0:c0 + cw], bank_of(m, 0, j)[sl, :cw])
                        elif heO:
                            nc.vector.tensor_copy(
                                stage[sl, c0:c0 + cw], bank_of(m, 1, j)[sl, :cw])
                        else:
                            nc.vector.memset(stage[sl, c0:c0 + cw], 0.0)
                    nc.sync.dma_start(
                        out_d[m * B:(m + 1) * B, 0:2 * CW], stage[0:64, 0:2 * CW])
                    nc.sync.dma_start(
                        out_d[m * B:(m + 1) * B, 2 * CW:NSH],
                        stage[64:128, 2 * CW:NSH])

                for ci, (r, m, k, start, stop, slot) in enumerate(cells):
                    if ci in drains:
                        for m2 in drains[ci]:
                            do_drain(m2)
                    xg = need_tile(slot)
                    col = (slot % G) * 64
                    t, rr = wloc[k]
                    assert rr == r
                    for j, (c0, cw) in enumerate(CHUNKS):
                        cc = j // 2
                        nc.tensor.matmul(
                            bank_of(m, r, j)[64 * cc:64 * cc + 64, :cw],
                            lhsT=xg[64 * r:64 * r + 64, col:col + 64],
                            rhs=wts[t][64 * r:64 * r + 64, c0:c0 + cw],
                            start=start, stop=stop,
                            skip_group_check=True,
                            tile_position=(64 * r, 64 * cc),
                        )
                if len(cells) in drains:
                    for m2 in drains[len(cells)]:
                        do_drain(m2)
    nc.compile()
    return nc


def _prepare(x, weight, reps=1):
    x = np.ascontiguousarray(np.asarray(x, dtype=np.float32))
    weight = np.ascontiguousarray(np.asarray(weight, dtype=np.float32))
    mask = _block_mask(x)
    par = _parity_assign(mask)
    qs, wloc, morder = _schedule(mask, par)
    cells, drains, nslots = _emit_order(qs, morder)
    xs, s_pad = _pack_stream(x, qs, morder, nslots)
    widx = _w_row_index(wloc)
    wperm = weight[widx].astype(ml_dtypes.bfloat16)
    in_maps = [
        {"xs": xs, "w": np.ascontiguousarray(wperm[:, c * NSH:(c + 1) * NSH])}
        for c in range(NCORES)
    ]
    nc = _build(qs, wloc, morder, cells, drains, s_pad, reps=reps)
    return nc, in_maps


def kernel(x, weight):
    nc, in_maps = _prepare(x, weight)
    res = run_bass_kernel_spmd(nc, in_maps, core_ids=list(range(NCORES)))
    out = np.concatenate([res.results[c]["out"] for c in range(NCORES)], axis=1)
    return np.ascontiguousarray(out)


# revision 3
# speedup vs baseline: 1.0177x; 1.0177x over previous
"""Block-sparse linear kernel for Trainium2 (8 NeuronCores) — quadrant design.

Computes: mask = mean|x| per 64x64 block > 0.798; out = (x*mask) @ weight
for x [4096,4096] f32, weight [4096,11008] f32 -> out [4096,11008] f32.

Strategy (v2 — PE 2x2 array tiling):
- Weight column-sharded across 8 cores (1376 cols each); x replicated.
- The PE array is addressed as four 64x64 quadrants via tile_position:
  row-half r = k-block parity (fixed by the w SBUF layout), col-half c =
  output chunk-group. N=1376 is split into 4 chunks of 344; quadrant
  (r, c) streams chunks {2c, 2c+1}. Each live (m,k) cell is ONE
  independent unit of work: 4 matmuls (2 per col-quadrant) sharing one
  [64,64] stationary x block. No m-block pairing, no union padding —
  every streamed column is live work, which is what beats the
  union-packed baseline (699 steps -> ~1003 cell-steps of half width).
- PSUM: per m-block, 8 accumulator half-banks (2 rows x 4 chunks) = 4
  banks; consecutive m-blocks flip-flop between bank sets {0..3} and
  {4..7}, so seat turnover never stalls the PE. Drains (ACT copy +
  DVE add per chunk; psum allows only one operand per instr) and 2
  output DMAs per m-block overlap the next block's accumulation.
- The E/O rows advance through the same m-order independently (kept
  within +-1 m by the bank flip-flop); per-m parity balance from the
  host-side assignment keeps that coupling cheap.
- bf16 inputs, fp32 PSUM accumulation. Host packs a compacted lhsT
  stream of live x blocks only (~16MB vs 25MB padded in v1).
"""

import numpy as np
import ml_dtypes

import concourse.bacc as bacc
import concourse.mybir as mybir
import concourse.tile as tile
from concourse.bass_utils import run_bass_kernel_spmd

M = 4096
K = 4096
N = 11008
B = 64
NB = M // B       # 64 blocks per dim
NCORES = 8
NSH = N // NCORES  # 1376 output cols per core
THRES = 0.798
CW = 344
CHUNKS = [(0, CW), (CW, CW), (2 * CW, CW), (3 * CW, CW)]
G = 32            # cells per x-stream DMA tile (per row-half)
BF16 = mybir.dt.bfloat16
F32 = mybir.dt.float32


def _block_mask(x):
    xb = np.abs(x.reshape(NB, B, NB, B))
    bm = xb.mean(axis=(1, 3), dtype=np.float64)
    return bm > THRES


def _parity_assign(mask):
    """Parity sig in {+-1} minimizing 8*|global E-O imbalance| (the wall
    time follows max(cellsE, cellsO)) + sum_m |ev_m - od_m| (keeps the two
    PE row-halves in lockstep per m)."""
    Mi = mask.astype(np.int32)
    rng = np.random.default_rng(1234)

    def cost(d):
        return np.abs(d).sum() + 8 * abs(int(d.sum()))

    best_sig, best_c = None, 1 << 30
    for _ in range(6):
        sig = np.array([1] * (NB // 2) + [-1] * (NB // 2))
        rng.shuffle(sig)
        improved = True
        while improved:
            improved = False
            d = Mi @ sig
            cur = cost(d)
            pos = np.where(sig == 1)[0]
            neg = np.where(sig == -1)[0]
            bestswap, bestdelta = None, 0
            for i in pos:
                for j in neg:
                    nd = d - 2 * Mi[:, i] + 2 * Mi[:, j]
                    delta = cost(nd) - cur
                    if delta < bestdelta:
                        bestdelta, bestswap = delta, (i, j)
            if bestswap is not None:
                i, j = bestswap
                sig[i], sig[j] = -1, 1
                improved = True
        c = cost(Mi @ sig)
        if c < best_c:
            best_c, best_sig = c, sig.copy()
    return (best_sig == -1).astype(np.int8)  # 1 = odd (array rows 64:128)


def _schedule(mask, par):
    """Per-m, per-parity k queues + w tile layout + m processing order."""
    evens = [b for b in range(NB) if par[b] == 0]
    odds = [b for b in range(NB) if par[b] == 1]
    assert len(evens) == len(odds) == NB // 2
    wloc = {}
    for t in range(NB // 2):
        wloc[evens[t]] = (t, 0)
        wloc[odds[t]] = (t, 1)
    morder = list(range(NB))
    qs = {}
    for m in range(NB):
        qs[(m, 0)] = [b for b in range(NB) if mask[m, b] and par[b] == 0]
        qs[(m, 1)] = [b for b in range(NB) if mask[m, b] and par[b] == 1]
    return qs, wloc, morder


def _emit_order(qs, morder):
    """Interleaved emission: alternate E/O cells, rows advance through the
    same m-order independently but stay within +-1 m (bank flip-flop).
    Returns: cells = list of (r, m, k, start, stop, slot) in program order,
    drains = {position_in_cells: [m, ...]} drains to emit after that cell,
    nslots = stream slots per row-half."""
    state = {r: {"mi": 0, "qi": 0} for r in (0, 1)}
    slots = {0: 0, 1: 0}
    done_m = {r: -1 for r in (0, 1)}  # last m-index fully emitted by row r
    cells = []
    drains = {}
    pend_drain = set()

    def row_can_emit(r):
        st = state[r]
        while st["mi"] < len(morder):
            m = morder[st["mi"]]
            q = qs[(m, r)]
            if st["qi"] < len(q):
                # bank constraint: row r may work on m-index i only if the
                # other row is at index >= i-1 (banks of i-2 must be free;
                # freed once both rows passed i-2 and drain emitted).
                other = 1 - r
                if st["mi"] - done_m[other] > 2:
                    return False
                return True
            done_m[r] = st["mi"]
            st["mi"] += 1
            st["qi"] = 0
            # m complete on this row; if complete on both -> drain
            if done_m[1 - r] >= st["mi"] - 1:
                pass
        return False

    def emit(r):
        st = state[r]
        m = morder[st["mi"]]
        q = qs[(m, r)]
        k = q[st["qi"]]
        start = st["qi"] == 0
        stop = st["qi"] == len(q) - 1
        cells.append((r, m, k, start, stop, slots[r]))
        slots[r] += 1
        st["qi"] += 1
        if stop:
            done_m[r] = st["mi"]
            st["mi"] += 1
            st["qi"] = 0
            # skip m's with an empty queue on this row
            while st["mi"] < len(morder) and not qs[(morder[st["mi"]], r)]:
                done_m[r] = st["mi"]
                st["mi"] += 1
        # emit drain for any m fully processed by both rows
        dm = min(done_m[0], done_m[1])
        for i in range(dm + 1):
            m2 = morder[i]
            if m2 not in pend_drain:
                pend_drain.add(m2)
                drains.setdefault(len(cells), []).append(m2)

    remaining = True
    turn = 0
    while remaining:
        e0 = row_can_emit(turn)
        e1 = row_can_emit(1 - turn)
        if e0:
            emit(turn)
        if e1:
            emit(1 - turn)
        if not e0 and not e1:
            if state[0]["mi"] >= len(morder) and state[1]["mi"] >= len(morder):
                remaining = False
            else:
                raise RuntimeError("emission deadlock")
        turn = 1 - turn
    # drain anything left
    tail = [m for m in morder if m not in pend_drain]
    if tail:
        drains.setdefault(len(cells), []).extend(tail)
    nslots = max(slots[0], slots[1])
    return cells, drains, nslots


def _pack_stream(x, qs, morder, nslots):
    """bf16 lhsT stream [128, S*64]: rows 0:64 = E cells (xT blocks in
    consumption order), rows 64:128 = O cells."""
    s_pad = ((nslots + G - 1) // G) * G
    xs = np.zeros((128, s_pad * 64), dtype=np.float32)
    for r in (0, 1):
        s = 0
        for m in morder:
            for k in qs[(m, r)]:
                blk = x[m * B:(m + 1) * B, k * B:(k + 1) * B]
                xs[64 * r:64 * r + 64, s * 64:(s + 1) * 64] = blk.T
                s += 1
    return xs.astype(ml_dtypes.bfloat16), s_pad


def _w_row_index(wloc):
    idx = np.empty(K, dtype=np.int64)
    for b, (t, r) in wloc.items():
        idx[128 * t + 64 * r: 128 * t + 64 * r + 64] = np.arange(b * B, (b + 1) * B)
    return idx


def _build(qs, wloc, morder, cells, drains, s_pad, reps=1):
    nc = bacc.Bacc()
    xs_d = nc.declare_dram_parameter("xs", [128, s_pad * 64], BF16, isOutput=False)
    w_d = nc.declare_dram_parameter("w", [K, NSH], BF16, isOutput=False)
    out_d = nc.declare_dram_parameter("out", [M, NSH], F32, isOutput=True)

    # w tile first-use order (preload in the order the schedule touches them)
    worder = []
    seen_t = set()
    for (r, m, k, st, sp, sl) in cells:
        t = wloc[k][0]
        if t not in seen_t:
            seen_t.add(t)
            worder.append(t)
    for t in range(NB // 2):
        if t not in seen_t:
            worder.append(t)

    mindex = {m: i for i, m in enumerate(morder)}

    with tile.TileContext(nc) as tc:
        with (
            tc.tile_pool(name="wp", bufs=1) as wp,
            tc.tile_pool(name="xp", bufs=6) as xp,
            tc.tile_pool(name="sp", bufs=6) as sp,
            tc.tile_pool(name="pp", bufs=1, space="PSUM") as pp,
        ):
            # w DMAs go on the scalar engine's queue so they don't head-block
            # the x-stream DMAs (sync queue); first-use order limits startup
            # stalls in the single-shot path.
            wts = {}
            for t in worder:
                wt = wp.tile([128, NSH], BF16, tag=f"w{t}", name=f"w{t}")
                nc.scalar.dma_start(wt[:], w_d[128 * t:128 * (t + 1), :])
                wts[t] = wt
            banks = [
                pp.tile([128, 512], F32, tag=f"b{i}", bufs=1, name=f"bank{i}")
                for i in range(8)
            ]

            def bank_of(m, r, j):
                bs = 4 * (mindex[m] % 2)
                return banks[bs + 2 * r + (j & 1)]

            for _ in range(reps):
                xg_tiles = {}
                stages = {}

                def need_tile(slot):
                    g = slot // G
                    if g not in xg_tiles:
                        xg = xp.tile([128, G * 64], BF16, tag="xg", name="xg")
                        nc.sync.dma_start(
                            xg[:], xs_d[:, g * G * 64:(g + 1) * G * 64])
                        xg_tiles[g] = xg
                    return xg_tiles[g]

                def do_drain(m):
                    heE = len(qs[(m, 0)]) > 0
                    heO = len(qs[(m, 1)]) > 0
                    stage = sp.tile([128, NSH], F32, tag="stage", name="stage")
                    for j, (c0, cw) in enumerate(CHUNKS):
                        cc = j // 2
                        sl = slice(64 * cc, 64 * cc + 64)
                        if heE and heO:
                            nc.scalar.copy(
                                stage[sl, c0:c0 + cw], bank_of(m, 0, j)[sl, :cw])
                            nc.vector.tensor_tensor(
                                stage[sl, c0:c0 + cw],
                                bank_of(m, 1, j)[sl, :cw],
                                stage[sl, c0:c0 + cw],
                                mybir.AluOpType.add)
                        elif heE:
                            nc.vector.tensor_copy(
                                stage[sl, c0:c0 + cw], bank_of(m, 0, j)[sl, :cw])
                        elif heO:
                            nc.vector.tensor_copy(
                                stage[sl, c0:c0 + cw], bank_of(m, 1, j)[sl, :cw])
                        else:
                            nc.vector.memset(stage[sl, c0:c0 + cw], 0.0)
                    nc.sync.dma_start(
                        out_d[m * B:(m + 1) * B, 0:2 * CW], stage[0:64, 0:2 * CW])
                    nc.sync.dma_start(
                        out_d[m * B:(m + 1) * B, 2 * CW:NSH],
                        stage[64:128, 2 * CW:NSH])

                for ci, (r, m, k, start, stop, slot) in enumerate(cells):
                    if ci in drains:
                        for m2 in drains[ci]:
                            do_drain(m2)
                    xg = need_tile(slot)
                    col = (slot % G) * 64
                    t, rr = wloc[k]
                    assert rr == r
                    for j, (c0, cw) in enumerate(CHUNKS):
                        cc = j // 2
                        nc.tensor.matmul(
                            bank_of(m, r, j)[64 * cc:64 * cc + 64, :cw],
                            lhsT=xg[64 * r:64 * r + 64, col:col + 64],
                            rhs=wts[t][64 * r:64 * r + 64, c0:c0 + cw],
                            start=start, stop=stop,
                            skip_group_check=True,
                            tile_position=(64 * r, 64 * cc),
                        )
                if len(cells) in drains:
                    for m2 in drains[len(cells)]:
                        do_drain(m2)
    nc.compile()
    return nc


def _prepare(x, weight, reps=1):
    x = np.ascontiguousarray(np.asarray(x, dtype=np.float32))
    weight = np.ascontiguousarray(np.asarray(weight, dtype=np.float32))
    mask = _block_mask(x)
    par = _parity_assign(mask)
    qs, wloc, morder = _schedule(mask, par)
    cells, drains, nslots = _emit_order(qs, morder)
    xs, s_pad = _pack_stream(x, qs, morder, nslots)
    widx = _w_row_index(wloc)
    wperm = weight[widx].astype(ml_dtypes.bfloat16)
    in_maps = [
        {"xs": xs, "w": np.ascontiguousarray(wperm[:, c * NSH:(c + 1) * NSH])}
        for c in range(NCORES)
    ]
    nc = _build(qs, wloc, morder, cells, drains, s_pad, reps=reps)
    return nc, in_maps


def kernel(x, weight):
    nc, in_maps = _prepare(x, weight)
    res = run_bass_kernel_spmd(nc, in_maps, core_ids=list(range(NCORES)))
    out = np.concatenate([res.results[c]["out"] for c in range(NCORES)], axis=1)
    return np.ascontiguousarray(out)
